# revision 8
# baseline (speedup 1.0000x reference)
"""Trainium2 Bass kernel for nn_LocalEnergyCore (sampling / local energy MLP).

Contract: kernel(**inputs) takes FULL unsharded inputs, returns FULL output
(scalar f32). Internally shards z along batch across 8 NeuronCores.

Per-core device program (B_loc = 512 samples):
  - Host pre-gathers each site's 3x3xK neighborhood, applies the per-site
    center-drop gather (so every site shares ONE plain W1), and packs fp8
    chunk tensors ctx [80, n_sites, 512]: partition r = ctx entry r
    (r=0..70 the 71 kept entries, r=71 a ones row folding b1 into the
    matmul, r=72..79 zero pad so the DMA descriptor fan-out stays sane).
  - ctx chunks stream in over BOTH DMA rings (HWDGE via sync + SWDGE via
    gpsimd) so the 16 SDMA engines stay fed; weights ride the scalar
    (ACT) HWDGE ring up front.
  - ~16 small dummy matmuls on zeroed data warm the PE HAM clock gate
    while the first chunk is in flight.
  - L1: 25 fp8 DoubleRow matmuls (one per site PAIR, shared block-diag
    stationary [80, 2, 128] = [Wx|0;0|Wx]) into 2-bank psum tiles
    [128, 1024]; 3 tiles rotate.
  - Relu + fp32->fp8 cast evacuates each 2-bank tile in ONE op
    (alternating DVE tensor_scalar_max / ACT activation Relu) into
    per-group h tiles [128, 2, 512] that are directly the L2 DR rhs.
  - L2: 12 fp8 DR matmuls accumulate logits into ONE [128, 512] psum
    bank. Group g's 4 sites land on out rows 8g..8g+3 so its stationary
    is a 16B-aligned 128-col window (offset 80-16*(g>>1)) of one of two
    [128, 2, 208] tensors (parity g&1 picks the tensor; the W2 strip
    sits at cols 80+8(g&1)..+3). The final odd pair uses a plain
    [128, 94] stationary -> rows 92/93. Unused rows accumulate zeros.
  - One DVE scalar_tensor_tensor: ((logit > -b2) != target) summed over
    batch -> counts [94, 1] (garbage rows compare 0 vs 0 -> 0); a tiny
    fp32 ones-matmul reduces counts to a [1, 1] psum scalar; DVE copies
    it to SBUF and a 4-byte DMA returns it. Host sums the 8 per-core
    totals and divides by B*S.
"""

import sys

for _p in ("/opt/trn_rl_repo",):
    if _p not in sys.path:
        sys.path.insert(0, _p)

import numpy as np
import ml_dtypes

B, K, H, W = 4096, 8, 64, 64
S, HID = 50, 64
NPAIR = S // 2
CTX = 9 * K - 1        # 71 kept ctx entries
R = 80                 # 71 ctx rows + ones row + 8 zero pad rows
N_CORES = 8
B_LOC = B // N_CORES
NROW = 94              # logit rows used: 8g+2q+r for g<12, 92/93 last pair
W2W = 208              # w2 window tensor width: offsets {0..80} + 128
NWARM = 10             # PE HAM warm-up dummy matmuls (N=256 each)

# ctx DMA chunk boundaries in pairs, and the HWDGE ring each chunk rides
# (SWDGE/gpsimd measured ~65 GB/s here - far too slow). The scalar ring
# carries w1 first (20 KB, needed immediately), then its chunks, then the
# remaining weights (needed from L2_0, ~2 us later) and targets (needed
# only at the final compare).
CHUNKS = [(0, 2, "sync"), (2, 8, "scalar"), (8, 14, "sync"),
          (14, 19, "scalar"), (19, 25, "sync")]

FP8 = ml_dtypes.float8_e4m3

LAST_RESULTS = None  # test harness introspection


def _row_of_site(s):
    """psum row for site s under the 8-spaced group layout."""
    if s >= 48:
        return 92 + (s - 48)
    return 8 * (s // 4) + (s % 4)


def _host_prep(z, W1, b1, W2, b2, b_idx, i_idx, j_idx):
    """Shard + lay out inputs; returns (in_maps, -b2)."""
    z = np.asarray(z, dtype=np.float32)
    W1 = np.asarray(W1, dtype=np.float32)
    b1 = np.asarray(b1, dtype=np.float32)
    W2 = np.asarray(W2, dtype=np.float32)
    b_idx = np.asarray(b_idx).astype(np.int64)
    i_idx = np.asarray(i_idx).astype(np.int64)
    j_idx = np.asarray(j_idx).astype(np.int64)

    di = np.repeat(np.array([-1, 0, 1]), 3)
    dj = np.tile(np.array([-1, 0, 1]), 3)
    ni = (i_idx[:, None] + di[None, :]) % H          # [S, 9]
    nj = (j_idx[:, None] + dj[None, :]) % W          # [S, 9]

    # [B, K, S, 9] -> ctx entries position-major, bit-minor: [B, S, 72],
    # then apply the center-drop gather exactly like the reference so the
    # device-side W1 is site-invariant.
    neigh = z[:, :, ni, nj]
    ctx_full = np.transpose(neigh, (0, 2, 3, 1)).reshape(B, S, 9 * K)
    drop = 4 * K + b_idx                             # [S]
    t = np.arange(CTX)
    gidx = t[None, :] + (t[None, :] >= drop[:, None])  # [S, CTX]
    ctx71 = np.take_along_axis(ctx_full, gidx[None, :, :], axis=2)  # [B,S,71]
    ctx8 = ctx71.astype(FP8)

    # shared L1 stationary [80, 2, 128], block-diagonal [Wx | 0; 0 | Wx]
    # where Wx = [W1; b1] on rows 0..71.
    Wx = np.zeros((R, HID), dtype=np.float32)
    Wx[:CTX] = W1
    Wx[CTX] = b1
    Wx8 = Wx.astype(FP8)
    w1_np = np.zeros((R, 2, 2 * HID), dtype=FP8)
    w1_np[:, 0, 0:HID] = Wx8
    w1_np[:, 1, HID:2 * HID] = Wx8

    # L2 window tensors [128, 2, 208] per group parity m: strip at big
    # cols 80+8m+2q+r; group g=2k+m slices cols [80-16k, 208-16k).
    W28 = W2.astype(FP8)
    w2t_np = np.zeros((2, 2 * HID, 2, W2W), dtype=FP8)
    for m in range(2):
        for q in range(2):
            w2t_np[m, 0:HID, q, 80 + 8 * m + 2 * q] = W28
            w2t_np[m, HID:2 * HID, q, 80 + 8 * m + 2 * q + 1] = W28
    # final odd pair: plain stationary -> rows 92, 93
    w2last_np = np.zeros((2 * HID, NROW), dtype=FP8)
    w2last_np[0:HID, 92] = W28
    w2last_np[HID:2 * HID, 93] = W28

    # targets permuted to the psum row layout; unused rows stay 0.
    row_of = np.array([_row_of_site(s) for s in range(S)])

    in_maps = []
    for c in range(N_CORES):
        bs = slice(c * B_LOC, (c + 1) * B_LOC)
        ctx_c = np.zeros((R, S, B_LOC), dtype=FP8)
        ctx_c[:CTX] = ctx8[bs].transpose(2, 1, 0)
        ctx_c[CTX] = np.float32(1.0)
        # fp8 targets (values 0/1, exact): 48 KB instead of 192 KB fp32
        targ_c = np.zeros((NROW, B_LOC), dtype=FP8)
        targ_c[row_of] = z[bs, b_idx, i_idx, j_idx].astype(FP8).T  # [50, 512]
        m = {"w1": w1_np, "w2t0": w2t_np[0], "w2t1": w2t_np[1],
             "w2last": w2last_np, "targ": targ_c}
        for ci, (p0, p1, _ring) in enumerate(CHUNKS):
            m[f"ctx{ci}"] = np.ascontiguousarray(ctx_c[:, 2 * p0:2 * p1, :])
        in_maps.append(m)
    return in_maps, -float(np.asarray(b2))


def _build_program(neg_b2):
    """Emit the per-core Bass program (identical across cores)."""
    import concourse.bacc as bacc
    import concourse.mybir as mybir
    import concourse.tile as tile

    fp32 = mybir.dt.float32
    fp8 = mybir.dt.float8e4
    DR = mybir.MatmulPerfMode.DoubleRow

    nc = bacc.Bacc("TRN2", target_bir_lowering=False, debug=False,
                   num_devices=N_CORES)

    ctx_d = []
    for ci, (p0, p1, _ring) in enumerate(CHUNKS):
        ctx_d.append(nc.dram_tensor(f"ctx{ci}", [R, 2 * (p1 - p0), B_LOC],
                                    fp8, kind="ExternalInput"))
    w1_d = nc.dram_tensor("w1", [R, 2, 2 * HID], fp8, kind="ExternalInput")
    w2t_d = [nc.dram_tensor(f"w2t{m}", [2 * HID, 2, W2W], fp8,
                            kind="ExternalInput") for m in range(2)]
    w2last_d = nc.dram_tensor("w2last", [2 * HID, NROW], fp8,
                              kind="ExternalInput")
    targ_d = nc.dram_tensor("targ", [NROW, B_LOC], fp8,
                            kind="ExternalInput")
    outp = nc.dram_tensor("out", [1, 1], fp32, kind="ExternalOutput")

    with tile.TileContext(nc) as tc:
        with (
            tc.tile_pool(name="const", bufs=1) as cpool,
            tc.tile_pool(name="hsb", bufs=13) as hpool,
            tc.tile_pool(name="scr", bufs=1, space="PSUM") as scrpool,
            tc.tile_pool(name="mega", bufs=3, space="PSUM") as megapool,
            tc.tile_pool(name="psl", bufs=1, space="PSUM") as pslpool,
        ):
            # --- tiny on-device constants -----------------------------
            dummy_sb = cpool.tile([R, 2, 256], fp8, tag="dummy")
            nc.vector.memset(dummy_sb[:, :, :], 0)
            ones_sb = cpool.tile([NROW, 1], fp32, tag="ones")
            nc.gpsimd.memset(ones_sb[:, :], 1.0)

            # --- input DMAs (all HWDGE; emission order = ring order) --
            ctx_sb = [cpool.tile([R, 2 * (p1 - p0), B_LOC], fp8,
                                 tag=f"ctx{ci}", name=f"ctx_sb{ci}")
                      for ci, (p0, p1, _r) in enumerate(CHUNKS)]
            ctx_t = [(ct, p0) for ct, (p0, p1, _r) in zip(ctx_sb, CHUNKS)]

            def dma_chunk(ci):
                ring = nc.sync if CHUNKS[ci][2] == "sync" else nc.scalar
                ring.dma_start(out=ctx_sb[ci][:, :, :].opt(),
                               in_=ctx_d[ci][:, :, :].opt())

            # sync ring: c0 first, then its later chunks
            dma_chunk(0)
            # scalar ring: w1 (20 KB, needed first) then chunk 1
            w1_sb = cpool.tile([R, 2, 2 * HID], fp8, tag="w1")
            nc.scalar.dma_start(out=w1_sb[:, :, :].opt(),
                                in_=w1_d[:, :, :].opt())
            dma_chunk(1)
            dma_chunk(2)            # sync
            # scalar ring: L2 weights (needed from ~round 2)
            w2t_sb = []
            for m in range(2):
                wt = cpool.tile([2 * HID, 2, W2W], fp8, tag=f"w2t{m}")
                nc.scalar.dma_start(out=wt[:, :, :].opt(),
                                    in_=w2t_d[m][:, :, :].opt())
                w2t_sb.append(wt)
            w2last_sb = cpool.tile([2 * HID, NROW], fp8, tag="w2last")
            nc.scalar.dma_start(out=w2last_sb[:, :], in_=w2last_d[:, :])
            dma_chunk(3)            # scalar
            dma_chunk(4)            # sync
            # targets (fp8): needed only at the final compare
            targ_sb = cpool.tile([NROW, B_LOC], fp8, tag="targ")
            nc.scalar.dma_start(out=targ_sb[:, :], in_=targ_d[:, :])

            logit_ps = pslpool.tile([2 * HID, B_LOC], fp32, tag="logit")

            # --- PE warm-up (HAM clock gate); logit bank is scratch
            # here, L2's start=True clears it later ---------------------
            for _ in range(NWARM):
                nc.tensor.matmul(logit_ps[0:128, 0:256],
                                 dummy_sb[:, :, 0:128], dummy_sb[:, :, :],
                                 start=True, stop=True, perf_mode=DR)

            # --- pair pipeline ----------------------------------------
            def pair_rhs(p):
                for ct, p0 in reversed(ctx_t):
                    if p >= p0:
                        i = p - p0
                        return ct[:, 2 * i:2 * i + 2, :]
                raise AssertionError

            h_sb = {}

            def emit_round(r):
                """L1 matmuls for pairs 2r, 2r+1 + one mega-evacuation."""
                mega = megapool.tile([2 * HID, 2 * B_LOC], fp32,
                                     tag="mega", name=f"mega{r}")
                for q in range(2):
                    p = 2 * r + q
                    nc.tensor.matmul(
                        mega[:, q * B_LOC:(q + 1) * B_LOC],
                        w1_sb[:, :, :], pair_rhs(p),
                        start=True, stop=True, perf_mode=DR)
                hg = hpool.tile([2 * HID, 2, B_LOC], fp8,
                                tag="hsb", name=f"hsb{r}")
                h_sb[r] = hg
                if r % 2 == 0:
                    nc.vector.tensor_scalar_max(
                        hg[:, :, :], mega[:, :], 0.0)
                else:
                    nc.scalar.activation(
                        out=hg[:, :, :], in_=mega[:, :],
                        func=mybir.ActivationFunctionType.Relu,
                        bias=0.0, scale=1.0)

            def emit_l2(g):
                off = 80 - 16 * (g >> 1)
                nc.tensor.matmul(
                    logit_ps[:, :],
                    w2t_sb[g & 1][:, :, off:off + 128],
                    h_sb[g][:, :, :],
                    start=(g == 0), stop=False, perf_mode=DR)

            # L2 for group g emitted two rounds after its evacuation so
            # the in-order PE queue never stalls on a lagging evac.
            for r in range(NPAIR // 2):          # rounds 0..11
                emit_round(r)
                if r >= 2:
                    emit_l2(r - 2)

            # final odd pair 24: plain matmul path -> rows 92/93
            mega_l = megapool.tile([2 * HID, 2 * B_LOC], fp32,
                                   tag="mega", name="megal")
            nc.tensor.matmul(mega_l[:, 0:B_LOC], w1_sb[:, :, :],
                             pair_rhs(NPAIR - 1),
                             start=True, stop=True, perf_mode=DR)
            h_last = hpool.tile([2 * HID, 2, B_LOC], fp8,
                                tag="hsb", name="hlast")
            nc.vector.tensor_scalar_max(h_last[:, 0, :], mega_l[:, 0:B_LOC],
                                        0.0)

            emit_l2(10)
            emit_l2(11)
            nc.tensor.matmul(logit_ps[0:NROW, :], w2last_sb[:, :],
                             h_last[:, 0, :], start=False, stop=True)

            # --- compare + reduce -------------------------------------
            junk = cpool.tile([NROW, B_LOC], fp32, tag="junk")
            counts = cpool.tile([NROW, 1], fp32, tag="counts")
            nc.vector.scalar_tensor_tensor(
                out=junk[:, :], in0=logit_ps[0:NROW, :], scalar=neg_b2,
                in1=targ_sb[:, :],
                op0=mybir.AluOpType.is_gt, op1=mybir.AluOpType.not_equal,
                accum_out=counts[:, :])

            # counts [94,1] -> scalar via fp32 ones-matmul (exact: integer
            # counts < 2^24), so the output DMA is a single 4-byte packet.
            sum_ps = scrpool.tile([1, 1], fp32, tag="sum")
            nc.tensor.matmul(sum_ps[:, :], counts[:, :], ones_sb[:, :],
                             start=True, stop=True)
            sum_sb = cpool.tile([1, 1], fp32, tag="sumsb")
            nc.vector.tensor_copy(sum_sb[:, :], sum_ps[:, :])

            nc.sync.dma_start(out=outp[:, :], in_=sum_sb[:, :])

    nc.compile()
    return nc


def kernel(**inputs):
    global LAST_RESULTS
    from concourse.bass_utils import run_bass_kernel_spmd

    in_maps, neg_b2 = _host_prep(
        inputs["z"], inputs["W1"], inputs["b1"], inputs["W2"],
        inputs["b2"], inputs["b_idx"], inputs["i_idx"], inputs["j_idx"])

    nc = _build_program(neg_b2)

    res = run_bass_kernel_spmd(nc, in_maps, list(range(N_CORES)))
    LAST_RESULTS = res
    total = 0.0
    for r in res.results:
        total += float(np.asarray(r["out"], dtype=np.float64)[0, 0])
    return np.float32(total / float(B * S))


# revision 10
# speedup vs baseline: 1.0017x; 1.0017x over previous
"""Trainium2 Bass kernel for nn_LocalEnergyCore (sampling / local energy MLP).

Contract: kernel(**inputs) takes FULL unsharded inputs, returns FULL output
(scalar f32). Internally shards z along batch across 8 NeuronCores.

Per-core device program (B_loc = 512 samples):
  - Host pre-gathers each site's 3x3xK neighborhood, applies the per-site
    center-drop gather (so every site shares ONE plain W1), and packs fp8
    chunk tensors ctx [80, n_sites, 512]: partition r = ctx entry r
    (r=0..70 the 71 kept entries, r=71 a ones row folding b1 into the
    matmul, r=72..79 zero pad so the DMA descriptor fan-out stays sane).
  - ctx chunks stream in over BOTH DMA rings (HWDGE via sync + SWDGE via
    gpsimd) so the 16 SDMA engines stay fed; weights ride the scalar
    (ACT) HWDGE ring up front.
  - ~16 small dummy matmuls on zeroed data warm the PE HAM clock gate
    while the first chunk is in flight.
  - L1: 25 fp8 DoubleRow matmuls (one per site PAIR, shared block-diag
    stationary [80, 2, 128] = [Wx|0;0|Wx]) into 2-bank psum tiles
    [128, 1024]; 3 tiles rotate.
  - Relu + fp32->fp8 cast evacuates each 2-bank tile in ONE op
    (alternating DVE tensor_scalar_max / ACT activation Relu) into
    per-group h tiles [128, 2, 512] that are directly the L2 DR rhs.
  - L2: 12 fp8 DR matmuls accumulate logits into ONE [128, 512] psum
    bank. Group g's 4 sites land on out rows 8g..8g+3 so its stationary
    is a 16B-aligned 128-col window (offset 80-16*(g>>1)) of one of two
    [128, 2, 208] tensors (parity g&1 picks the tensor; the W2 strip
    sits at cols 80+8(g&1)..+3). The final odd pair uses a plain
    [128, 94] stationary -> rows 92/93. Unused rows accumulate zeros.
  - One DVE scalar_tensor_tensor: ((logit > -b2) != target) summed over
    batch -> counts [94, 1] (garbage rows compare 0 vs 0 -> 0); a tiny
    fp32 ones-matmul reduces counts to a [1, 1] psum scalar; DVE copies
    it to SBUF and a 4-byte DMA returns it. Host sums the 8 per-core
    totals and divides by B*S.
"""

import sys

for _p in ("/opt/trn_rl_repo",):
    if _p not in sys.path:
        sys.path.insert(0, _p)

import numpy as np
import ml_dtypes

B, K, H, W = 4096, 8, 64, 64
S, HID = 50, 64
NPAIR = S // 2
CTX = 9 * K - 1        # 71 kept ctx entries
R = 80                 # 71 ctx rows + ones row + 8 zero pad rows
N_CORES = 8
B_LOC = B // N_CORES
NROW = 94              # logit rows used: 8g+2q+r for g<12, 92/93 last pair
W2W = 208              # w2 window tensor width: offsets {0..80} + 128
NWARM = 10             # PE HAM warm-up dummy matmuls (N=256 each)

# ctx DMA chunk boundaries in pairs, and the ring each chunk rides.
# Fine-grained sync/scalar alternation keeps global arrival order close
# to the in-order PE consumption; the final chunk + targets ride the
# otherwise-idle gpsimd SWDGE ring (slow but prefetched far ahead).
CHUNKS = [(0, 2, "sync"), (2, 4, "scalar"), (4, 6, "sync"),
          (6, 8, "scalar"), (8, 11, "sync"), (11, 14, "scalar"),
          (14, 17, "sync"), (17, 19, "scalar"), (19, 22, "sync"),
          (22, 25, "gpsimd")]

FP8 = ml_dtypes.float8_e4m3

LAST_RESULTS = None  # test harness introspection


def _row_of_site(s):
    """psum row for site s under the 8-spaced group layout."""
    if s >= 48:
        return 92 + (s - 48)
    return 8 * (s // 4) + (s % 4)


def _host_prep(z, W1, b1, W2, b2, b_idx, i_idx, j_idx):
    """Shard + lay out inputs; returns (in_maps, -b2)."""
    z = np.asarray(z, dtype=np.float32)
    W1 = np.asarray(W1, dtype=np.float32)
    b1 = np.asarray(b1, dtype=np.float32)
    W2 = np.asarray(W2, dtype=np.float32)
    b_idx = np.asarray(b_idx).astype(np.int64)
    i_idx = np.asarray(i_idx).astype(np.int64)
    j_idx = np.asarray(j_idx).astype(np.int64)

    di = np.repeat(np.array([-1, 0, 1]), 3)
    dj = np.tile(np.array([-1, 0, 1]), 3)
    ni = (i_idx[:, None] + di[None, :]) % H          # [S, 9]
    nj = (j_idx[:, None] + dj[None, :]) % W          # [S, 9]

    # [B, K, S, 9] -> ctx entries position-major, bit-minor: [B, S, 72],
    # then apply the center-drop gather exactly like the reference so the
    # device-side W1 is site-invariant.
    neigh = z[:, :, ni, nj]
    ctx_full = np.transpose(neigh, (0, 2, 3, 1)).reshape(B, S, 9 * K)
    drop = 4 * K + b_idx                             # [S]
    t = np.arange(CTX)
    gidx = t[None, :] + (t[None, :] >= drop[:, None])  # [S, CTX]
    ctx71 = np.take_along_axis(ctx_full, gidx[None, :, :], axis=2)  # [B,S,71]
    ctx8 = ctx71.astype(FP8)

    # shared L1 stationary [80, 2, 128], block-diagonal [Wx | 0; 0 | Wx]
    # where Wx = [W1; b1] on rows 0..71.
    Wx = np.zeros((R, HID), dtype=np.float32)
    Wx[:CTX] = W1
    Wx[CTX] = b1
    Wx8 = Wx.astype(FP8)
    w1_np = np.zeros((R, 2, 2 * HID), dtype=FP8)
    w1_np[:, 0, 0:HID] = Wx8
    w1_np[:, 1, HID:2 * HID] = Wx8

    # L2 window tensors [128, 2, 208] per group parity m: strip at big
    # cols 80+8m+2q+r; group g=2k+m slices cols [80-16k, 208-16k).
    W28 = W2.astype(FP8)
    w2t_np = np.zeros((2, 2 * HID, 2, W2W), dtype=FP8)
    for m in range(2):
        for q in range(2):
            w2t_np[m, 0:HID, q, 80 + 8 * m + 2 * q] = W28
            w2t_np[m, HID:2 * HID, q, 80 + 8 * m + 2 * q + 1] = W28
    # final odd pair: plain stationary -> rows 92, 93
    w2last_np = np.zeros((2 * HID, NROW), dtype=FP8)
    w2last_np[0:HID, 92] = W28
    w2last_np[HID:2 * HID, 93] = W28

    # targets permuted to the psum row layout; unused rows stay 0.
    row_of = np.array([_row_of_site(s) for s in range(S)])

    in_maps = []
    for c in range(N_CORES):
        bs = slice(c * B_LOC, (c + 1) * B_LOC)
        ctx_c = np.zeros((R, S, B_LOC), dtype=FP8)
        ctx_c[:CTX] = ctx8[bs].transpose(2, 1, 0)
        ctx_c[CTX] = np.float32(1.0)
        # fp8 targets (values 0/1, exact): 48 KB instead of 192 KB fp32
        targ_c = np.zeros((NROW, B_LOC), dtype=FP8)
        targ_c[row_of] = z[bs, b_idx, i_idx, j_idx].astype(FP8).T  # [50, 512]
        m = {"w1": w1_np, "w2t0": w2t_np[0], "w2t1": w2t_np[1],
             "w2last": w2last_np, "targ": targ_c}
        for ci, (p0, p1, _ring) in enumerate(CHUNKS):
            m[f"ctx{ci}"] = np.ascontiguousarray(ctx_c[:, 2 * p0:2 * p1, :])
        in_maps.append(m)
    return in_maps, -float(np.asarray(b2))


def _build_program(neg_b2):
    """Emit the per-core Bass program (identical across cores)."""
    import concourse.bacc as bacc
    import concourse.mybir as mybir
    import concourse.tile as tile

    fp32 = mybir.dt.float32
    fp8 = mybir.dt.float8e4
    DR = mybir.MatmulPerfMode.DoubleRow

    nc = bacc.Bacc("TRN2", target_bir_lowering=False, debug=False,
                   num_devices=N_CORES)

    ctx_d = []
    for ci, (p0, p1, _ring) in enumerate(CHUNKS):
        ctx_d.append(nc.dram_tensor(f"ctx{ci}", [R, 2 * (p1 - p0), B_LOC],
                                    fp8, kind="ExternalInput"))
    w1_d = nc.dram_tensor("w1", [R, 2, 2 * HID], fp8, kind="ExternalInput")
    w2t_d = [nc.dram_tensor(f"w2t{m}", [2 * HID, 2, W2W], fp8,
                            kind="ExternalInput") for m in range(2)]
    w2last_d = nc.dram_tensor("w2last", [2 * HID, NROW], fp8,
                              kind="ExternalInput")
    targ_d = nc.dram_tensor("targ", [NROW, B_LOC], fp8,
                            kind="ExternalInput")
    outp = nc.dram_tensor("out", [1, 1], fp32, kind="ExternalOutput")

    with tile.TileContext(nc) as tc:
        with (
            tc.tile_pool(name="const", bufs=1) as cpool,
            tc.tile_pool(name="hsb", bufs=13) as hpool,
            tc.tile_pool(name="scr", bufs=1, space="PSUM") as scrpool,
            tc.tile_pool(name="mega", bufs=3, space="PSUM") as megapool,
            tc.tile_pool(name="psl", bufs=1, space="PSUM") as pslpool,
        ):
            # --- tiny on-device constants -----------------------------
            dummy_sb = cpool.tile([R, 2, 256], fp8, tag="dummy")
            nc.vector.memset(dummy_sb[:, :, :], 0)
            ones_sb = cpool.tile([NROW, 1], fp32, tag="ones")
            nc.gpsimd.memset(ones_sb[:, :], 1.0)

            # --- input DMAs (emission order = per-ring FIFO order) -----
            ctx_sb = [cpool.tile([R, 2 * (p1 - p0), B_LOC], fp8,
                                 tag=f"ctx{ci}", name=f"ctx_sb{ci}")
                      for ci, (p0, p1, _r) in enumerate(CHUNKS)]
            ctx_t = [(ct, p0) for ct, (p0, p1, _r) in zip(ctx_sb, CHUNKS)]
            rings = {"sync": nc.sync, "scalar": nc.scalar,
                     "gpsimd": nc.gpsimd}

            def dma_chunk(ci):
                rings[CHUNKS[ci][2]].dma_start(
                    out=ctx_sb[ci][:, :, :].opt(),
                    in_=ctx_d[ci][:, :, :].opt())

            # sync ring: w1 (20 KB, needed first) then chunk 0
            w1_sb = cpool.tile([R, 2, 2 * HID], fp8, tag="w1")
            nc.sync.dma_start(out=w1_sb[:, :, :].opt(),
                              in_=w1_d[:, :, :].opt())
            dma_chunk(0)            # sync
            dma_chunk(1)            # scalar
            dma_chunk(2)            # sync
            dma_chunk(3)            # scalar
            dma_chunk(4)            # sync
            # scalar ring: L2 weights (needed from ~round 2)
            w2t_sb = []
            for m in range(2):
                wt = cpool.tile([2 * HID, 2, W2W], fp8, tag=f"w2t{m}")
                nc.scalar.dma_start(out=wt[:, :, :].opt(),
                                    in_=w2t_d[m][:, :, :].opt())
                w2t_sb.append(wt)
            w2last_sb = cpool.tile([2 * HID, NROW], fp8, tag="w2last")
            nc.scalar.dma_start(out=w2last_sb[:, :], in_=w2last_d[:, :])
            # gpsimd SWDGE: slow ring, but its chunk is needed last and
            # its descriptor generation rides an otherwise-idle engine
            dma_chunk(9)            # gpsimd
            targ_sb = cpool.tile([NROW, B_LOC], fp8, tag="targ")
            nc.gpsimd.dma_start(out=targ_sb[:, :], in_=targ_d[:, :])
            dma_chunk(5)            # scalar
            dma_chunk(6)            # sync
            dma_chunk(7)            # scalar
            dma_chunk(8)            # sync

            logit_ps = pslpool.tile([2 * HID, B_LOC], fp32, tag="logit")

            # --- PE warm-up (HAM clock gate); logit bank is scratch
            # here, L2's start=True clears it later ---------------------
            for _ in range(NWARM):
                nc.tensor.matmul(logit_ps[0:128, 0:256],
                                 dummy_sb[:, :, 0:128], dummy_sb[:, :, :],
                                 start=True, stop=True, perf_mode=DR)

            # --- pair pipeline ----------------------------------------
            def pair_rhs(p):
                for ct, p0 in reversed(ctx_t):
                    if p >= p0:
                        i = p - p0
                        return ct[:, 2 * i:2 * i + 2, :]
                raise AssertionError

            h_sb = {}

            def emit_round(r):
                """L1 matmuls for pairs 2r, 2r+1 + one mega-evacuation."""
                mega = megapool.tile([2 * HID, 2 * B_LOC], fp32,
                                     tag="mega", name=f"mega{r}")
                for q in range(2):
                    p = 2 * r + q
                    nc.tensor.matmul(
                        mega[:, q * B_LOC:(q + 1) * B_LOC],
                        w1_sb[:, :, :], pair_rhs(p),
                        start=True, stop=True, perf_mode=DR)
                hg = hpool.tile([2 * HID, 2, B_LOC], fp8,
                                tag="hsb", name=f"hsb{r}")
                h_sb[r] = hg
                if r % 2 == 0:
                    nc.vector.tensor_scalar_max(
                        hg[:, :, :], mega[:, :], 0.0)
                else:
                    nc.scalar.activation(
                        out=hg[:, :, :], in_=mega[:, :],
                        func=mybir.ActivationFunctionType.Relu,
                        bias=0.0, scale=1.0)

            def emit_l2(g):
                off = 80 - 16 * (g >> 1)
                nc.tensor.matmul(
                    logit_ps[:, :],
                    w2t_sb[g & 1][:, :, off:off + 128],
                    h_sb[g][:, :, :],
                    start=(g == 0), stop=False, perf_mode=DR)

            # L2 for group g emitted two rounds after its evacuation so
            # the in-order PE queue never stalls on a lagging evac.
            for r in range(NPAIR // 2):          # rounds 0..11
                emit_round(r)
                if r >= 2:
                    emit_l2(r - 2)

            # final odd pair 24: plain matmul path -> rows 92/93
            mega_l = megapool.tile([2 * HID, 2 * B_LOC], fp32,
                                   tag="mega", name="megal")
            nc.tensor.matmul(mega_l[:, 0:B_LOC], w1_sb[:, :, :],
                             pair_rhs(NPAIR - 1),
                             start=True, stop=True, perf_mode=DR)
            h_last = hpool.tile([2 * HID, 2, B_LOC], fp8,
                                tag="hsb", name="hlast")
            nc.vector.tensor_scalar_max(h_last[:, 0, :], mega_l[:, 0:B_LOC],
                                        0.0)

            emit_l2(10)
            emit_l2(11)
            nc.tensor.matmul(logit_ps[0:NROW, :], w2last_sb[:, :],
                             h_last[:, 0, :], start=False, stop=True)

            # --- compare + reduce -------------------------------------
            junk = cpool.tile([NROW, B_LOC], fp32, tag="junk")
            counts = cpool.tile([NROW, 1], fp32, tag="counts")
            nc.vector.scalar_tensor_tensor(
                out=junk[:, :], in0=logit_ps[0:NROW, :], scalar=neg_b2,
                in1=targ_sb[:, :],
                op0=mybir.AluOpType.is_gt, op1=mybir.AluOpType.not_equal,
                accum_out=counts[:, :])

            # counts [94,1] -> scalar via fp32 ones-matmul (exact: integer
            # counts < 2^24), so the output DMA is a single 4-byte packet.
            sum_ps = scrpool.tile([1, 1], fp32, tag="sum")
            nc.tensor.matmul(sum_ps[:, :], counts[:, :], ones_sb[:, :],
                             start=True, stop=True)
            sum_sb = cpool.tile([1, 1], fp32, tag="sumsb")
            nc.vector.tensor_copy(sum_sb[:, :], sum_ps[:, :])

            nc.sync.dma_start(out=outp[:, :], in_=sum_sb[:, :])

    nc.compile()
    return nc


def kernel(**inputs):
    global LAST_RESULTS
    from concourse.bass_utils import run_bass_kernel_spmd

    in_maps, neg_b2 = _host_prep(
        inputs["z"], inputs["W1"], inputs["b1"], inputs["W2"],
        inputs["b2"], inputs["b_idx"], inputs["i_idx"], inputs["j_idx"])

    nc = _build_program(neg_b2)

    res = run_bass_kernel_spmd(nc, in_maps, list(range(N_CORES)))
    LAST_RESULTS = res
    total = 0.0
    for r in res.results:
        total += float(np.asarray(r["out"], dtype=np.float64)[0, 0])
    return np.float32(total / float(B * S))
